# revision 6
# baseline (speedup 1.0000x reference)
"""KCompetitive (k_comp_tanh training branch) Trainium2 kernel.

Per row of x [16384, 2048]:
  P = relu(x), N = min(x, 0); the top-32 of P and of -N are "winners".
  Loser energy of each sign is amplified by FACTOR and added onto the
  winners; everything else is zeroed:
    out[j] = x[j] + P_tmp   if x[j] in top-32 positives
    out[j] = x[j] - N_tmp   if x[j] in top-32 magnitudes of negatives
    out[j] = 0              otherwise
  with P_tmp = FACTOR * (sum(P) - sum(top32(P))), N_tmp likewise.

Sharding: rows are data-parallel across 8 NeuronCores (2048 rows/core),
processed in 16 tiles of [128 partitions, 2048] per core.

Selection per side uses DVE max (top-8 per partition) + match_replace
(replace those 8 with 0.0), 4 rounds => top-32, on a scratch copy of the
relu buffer. Winners are recovered positionally as
  w_p = relu(x) - destroyed_buffer   (= x at winner positions, else 0)
which reproduces jax.lax.top_k's lowest-index tie-break for duplicate
values (match_replace replaces one occurrence per entry).
Output: out = (w_p + [w_p>0]*P_tmp) - (w_n + [w_n>0]*N_tmp).
relu + row sums run fused on the Scalar engine; the compare*scale is a
single fused DVE tensor_scalar; the negative-side combines are offloaded
to GpSimd so DVE stays on the selection critical path.
"""

import sys

sys.path.insert(0, "/opt/trn_rl_repo")

import numpy as np

import concourse.bacc as bacc
import concourse.mybir as mybir
from concourse.bass_utils import run_bass_kernel_spmd
from concourse.tile import TileContext

AF = mybir.ActivationFunctionType
ALU = mybir.AluOpType
F32 = mybir.dt.float32
AX = mybir.AxisListType

N_CORES = 8
ROWS, COLS = 16384, 2048
RPC = ROWS // N_CORES  # rows per core
P = 128  # SBUF partitions
NTILES = RPC // P
FACTOR = 6.26
K = 32  # winners per sign

_NC_CACHE = {}


def _select_topk(nc, sp, src, scratch, k):
    """Top-k (k % 8 == 0) per partition of `src` (read-only). `scratch`
    ends as a copy of src with the k winners replaced by 0.0. Returns a
    [P, k] tile of winner values in descending order."""
    mx = sp.tile([P, k], F32)
    work = src
    for r in range(k // 8):
        sl = mx[:, r * 8 : (r + 1) * 8]
        nc.vector.max(out=sl, in_=work)
        nc.vector.match_replace(
            out=scratch, in_to_replace=sl, in_values=work, imm_value=0.0
        )
        work = scratch
    return mx


def _build_program():
    # Bacc (not raw Bass): its compile() runs generate_event_semaphores,
    # which splits multi-wait instructions to satisfy the TRN2 limit of
    # one sync wait per instruction.
    nc = bacc.Bacc()
    x_d = nc.declare_dram_parameter("x", [RPC, COLS], F32, isOutput=False)
    o_d = nc.declare_dram_parameter("out", [RPC, COLS], F32, isOutput=True)

    with TileContext(nc) as tc:
        with (
            tc.tile_pool(name="big", bufs=2) as pool,
            tc.tile_pool(name="small", bufs=3) as sp,
        ):
            for t in range(NTILES):
                rs = slice(t * P, (t + 1) * P)
                xt = pool.tile([P, COLS], F32)
                nc.sync.dma_start(out=xt, in_=x_d[rs])

                # relu(+-x) with fused row sums on ACT.
                rp = pool.tile([P, COLS], F32)
                sump = sp.tile([P, 1], F32)
                nc.scalar.activation(out=rp, in_=xt, func=AF.Relu, accum_out=sump)
                rm = pool.tile([P, COLS], F32)
                summ = sp.tile([P, 1], F32)
                nc.scalar.activation(
                    out=rm, in_=xt, func=AF.Relu, scale=-1.0, accum_out=summ
                )

                rp2 = pool.tile([P, COLS], F32)
                mxp = _select_topk(nc, sp, rp, rp2, K)
                rm2 = pool.tile([P, COLS], F32)
                mxm = _select_topk(nc, sp, rm, rm2, K)

                # ptmp = FACTOR * (sum_P - winner_sum_p); ntmp likewise.
                wsp = sp.tile([P, 1], F32)
                nc.vector.reduce_sum(out=wsp, in_=mxp, axis=AX.X)
                wsm = sp.tile([P, 1], F32)
                nc.vector.reduce_sum(out=wsm, in_=mxm, axis=AX.X)
                ptmp = sp.tile([P, 1], F32)
                nc.vector.tensor_scalar(
                    out=ptmp, in0=sump, scalar1=wsp, scalar2=FACTOR,
                    op0=ALU.subtract, op1=ALU.mult,
                )
                ntmp = sp.tile([P, 1], F32)
                nc.vector.tensor_scalar(
                    out=ntmp, in0=summ, scalar1=wsm, scalar2=FACTOR,
                    op0=ALU.subtract, op1=ALU.mult,
                )

                # Winner values by position; add the per-row amplification on
                # winner positions only.
                wp = pool.tile([P, COLS], F32)
                nc.vector.tensor_sub(wp, rp, rp2)
                wn = pool.tile([P, COLS], F32)
                nc.gpsimd.tensor_sub(wn, rm, rm2)

                up = pool.tile([P, COLS], F32)
                nc.vector.tensor_scalar(
                    out=up, in0=wp, scalar1=0.0, scalar2=ptmp,
                    op0=ALU.is_gt, op1=ALU.mult,
                )
                un = pool.tile([P, COLS], F32)
                nc.vector.tensor_scalar(
                    out=un, in0=wn, scalar1=0.0, scalar2=ntmp,
                    op0=ALU.is_gt, op1=ALU.mult,
                )

                a = pool.tile([P, COLS], F32)
                nc.vector.tensor_add(a, wp, up)
                b = pool.tile([P, COLS], F32)
                nc.gpsimd.tensor_add(b, wn, un)
                ot = pool.tile([P, COLS], F32)
                nc.vector.tensor_sub(ot, a, b)

                nc.sync.dma_start(out=o_d[rs], in_=ot)
    # Bacc.finalize runs compile(): register allocation + the
    # generate_event_semaphores legalization (<=1 sync wait per inst).
    nc.finalize()
    return nc


def _get_program():
    if "nc" not in _NC_CACHE:
        _NC_CACHE["nc"] = _build_program()
    return _NC_CACHE["nc"]


def kernel(x: np.ndarray) -> np.ndarray:
    x = np.ascontiguousarray(np.asarray(x), dtype=np.float32)
    assert x.shape == (ROWS, COLS), x.shape
    nc = _get_program()
    shards = np.split(x, N_CORES, axis=0)
    in_maps = [{"x": s} for s in shards]
    res = run_bass_kernel_spmd(nc, in_maps, core_ids=list(range(N_CORES)))
    return np.concatenate([r["out"] for r in res.results], axis=0)
